# revision 1
# baseline (speedup 1.0000x reference)
"""Multi-head attention (B=1, S=4096, H=16, D=64) on 8 Trainium2 NeuronCores.

Sharding: 2 heads per core (pure head-parallel, no cross-core comms).

The kernel is ScalarE-throughput-bound at heart: softmax needs 33.6M exps per
core and the Activation engine sustains ~1ns/col (~285us ceiling), which also
paced the original pipeline.  This version offloads ~23% of the exp tiles to
the Vector engine via a 3-instruction exp-to-bf16-bits pipeline:

  J = u16(psum_scores * (128*log2e/8) + c1b)   # Schraudolph: linear-mantissa
                                               #   bf16 bit pattern (tensor_scalar)
  I = i16(J * 1/128 - 63.496)                  # octave index from J; off-by-one
                                               #   only where the correction -> 0
  P = u16(J - g*w*(128-w)),  w = J - 128*I + c2  # custom fused DVE op (7 ALU
                                               #   stages): quadratic mantissa fix

P bitcast to bf16 equals exp(s/8)/16 within 0.9% max / 0.24% mean (validated
on hw; hw float->int converts round-to-nearest, the constants assume it).
Per-element exp error is zero-mean and softmax normalization absorbs the
common bias.  ScalarE handles the remaining tiles with exact table-based Exp
at [128,1024] tiles.  Only the J pass touches PSUM, so a DVE-exp chunk holds
its score tile barely longer than a ScalarE chunk.

Other structure kept from the bf16 baseline: scores computed transposed
(psT[kk,qq]) so exp tiles feed PV's moving operand directly; V carries a ones
column so PV row 64 accumulates softmax denominators; drains PE-transpose
[65,128] slices back and normalize on DVE.

Scheduling notes (all measured on hw):
- All PSUM transpose scratch (prep transposes, drain tp2, warmup) shares the
  score pool's tag, giving a 3-slot x [128,1024] rotation (6 banks) + 2
  accumulator banks = 8.  With 3 slots the QK->exp->slot-free recycle has a
  full step of slack, which removed ~0.6us/step of just-in-time semaphore
  stalls (the 2-slot version ran ~14us slower).
- DVE-exp chunks sit 4 apart (8 per superblock; fewer in the DMA-heavy first
  superblock); their J/I passes are emitted one step early.
- K/V prep blocks beyond the first two interleave into the early main-loop
  steps so the first exp starts ~10us sooner.
- PV emission is out-of-order-ready (a late DVE tile never blocks ready act
  PVs); h1 PVs trail h0 by one step; drains run every 4th step.
"""

import sys

for _p in ("/opt/trn_rl_repo", "/root/.axon_site/_ro/trn_rl_repo"):
    if _p not in sys.path:
        sys.path.append(_p)

import numpy as np

_B, _S, _H, _D = 1, 4096, 16, 64
_NCORES = 8
_HPC = _H // _NCORES  # heads per core

_LOG2E = float(np.log2(np.e))
_SCALE = 0.125            # 1/sqrt(D)
_EXPB = -float(np.log(16.0))  # store probs as exp(s)/16 (fp16 drain headroom)
_A1 = _SCALE * _LOG2E
_Q1 = 0.66
_GAMMA = (1.0 - _Q1) / 128.0


def _dve_consts(hw_round=True):
    eb2 = _EXPB * _LOG2E
    rfix = -0.5 if hw_round else 0.0
    c1a = 64.0 + eb2 + rfix
    ca = 128.0 * _A1
    c1b = 128.0 * (eb2 + 127.0) + (0.0 if hw_round else 0.5)
    c2 = -8064.0
    return c1a, ca, c1b, c2


def register_pass2_op():
    import concourse.dve_ops as dve_ops
    from concourse.dve_ops import DveOp
    from concourse.dve_spec import Spec, Src0, Src1, C0, C1, C2, AluOp, Bin, lower
    from concourse.dve_uop import DveOpSpec

    name = "EXPFIX_ANT"
    if name in dve_ops._SUB_OPCODE_FOR_NAME:
        return next(op for op in dve_ops.OPS if op.name == name)
    mult, add, sub = AluOp.MULTIPLY, AluOp.ADD, AluOp.SUBTRACT
    t = Bin(mult, Src0, C0)      # 128*I
    u = Bin(sub, Src1, t)        # J - 128*I
    w = Bin(add, u, C1)          # ~ 128*frac(y)
    g = Bin(sub, C0, w)          # 128 - w
    h = Bin(mult, g, w)
    G = Bin(mult, h, C2)         # * -gamma
    bits = Bin(add, Src1, G)

    def ref(in0, in1, c0, c1, c2):
        a0 = np.asarray(in0, np.float32)
        a1 = np.asarray(in1, np.float32)
        w = (a1 - a0 * c0) + c1
        return (a1 + (c0 - w) * w * c2).astype(np.float32)

    spec = Spec(body=bits, reference=ref)
    row = max(dve_ops._SUB_OPCODE_FOR_NAME.values()) + 1
    dve_ops._SUB_OPCODE_FOR_NAME[name] = row
    uops = lower(spec, ver="v3")
    sha = DveOpSpec(name=name, opcode=row, uops=uops, rd1_en=True).sha("v3")
    op = DveOp(name, spec, subdim=False, uops_sha={"v3": sha}, perf_en={"v3": True})
    dve_ops.OPS.append(op)
    dve_ops.CUSTOM_DVE_SPECS[name] = spec
    return op


def build_program(S=_S, n_heads=_HPC, blk=512, hw_round=True, dve_cs=None,
                  dve_cs_b0=None):
    """Single-core Bass program (SPMD).

    Score tiles are per-(chunk, head) [128, 512] f32 = one PSUM bank each
    (PSUM accumulation groups are 2KB-bank granular).  ScalarE tiles rotate
    3 banks; DVE-exp chunks get a dedicated [128, 1024] 2-bank tile so the
    slower 3-pass DVE pipeline never stalls the QK stream."""
    import concourse.tile as tile
    from concourse import bacc, mybir
    from concourse.masks import make_identity

    f32 = mybir.dt.float32
    bf16 = mybir.dt.bfloat16
    f16 = mybir.dt.float16
    i16 = mybir.dt.int16
    u16 = mybir.dt.uint16
    D = _D
    W = n_heads * D
    n_sk = S // 128
    n_blk = S // blk
    n_j = blk // 128
    assert n_heads == 2 and W == 128 and blk % 128 == 0

    exp_op = register_pass2_op()
    c1a, ca, c1b, c2 = _dve_consts(hw_round)

    if dve_cs is None:
        dve_cs = (1, 5, 9, 13, 17, 21, 25, 29) if S == _S else ()
    if dve_cs_b0 is None:
        dve_cs_b0 = (17, 21, 25, 29) if S == _S else ()

    nc = bacc.Bacc("TRN2", target_bir_lowering=False, debug=False)
    q_in = nc.dram_tensor("q", [S, W], f32, kind="ExternalInput")
    k_in = nc.dram_tensor("k", [S, W], f32, kind="ExternalInput")
    v_in = nc.dram_tensor("v", [S, W], f32, kind="ExternalInput")
    out = nc.dram_tensor("out", [S, W], f32, kind="ExternalOutput")

    with tile.TileContext(nc) as tc:
        with (
            tc.tile_pool(name="singles", bufs=1) as singles,
            tc.tile_pool(name="ld", bufs=4) as ld,
            tc.tile_pool(name="qkt", bufs=1) as qkt,
            tc.tile_pool(name="vp", bufs=1) as vpp,
            tc.tile_pool(name="expool", bufs=12) as expool,
            tc.tile_pool(name="ijpool", bufs=2) as ijpool,
            tc.tile_pool(name="osb", bufs=3) as osb,
            tc.tile_pool(name="outb", bufs=3) as outb,
            tc.tile_pool(name="small", bufs=4) as small,
            tc.tile_pool(name="ps_a", bufs=3, space="PSUM") as ps_act,
            tc.tile_pool(name="ps_o", bufs=1, space="PSUM") as ps_out,
        ):
            ident128_bf = singles.tile([128, 128], bf16)
            make_identity(nc, ident128_bf)
            ident65 = singles.tile([65, 65], f16)
            make_identity(nc, ident65)

            exp_bias = singles.tile([128, 1], f32, tag="expb")
            nc.vector.memset(exp_bias, _EXPB)

            dum = small.tile([128, 1], f32, tag="rec", name="dum")
            nc.vector.memset(dum, 0.0)
            dum2 = small.tile([128, 1], f32, tag="rec", name="dum2")
            nc.scalar.activation(dum2, dum, mybir.ActivationFunctionType.Exp)

            warm = ps_act.tile([128, 128], bf16, tag="psa", name="warm")
            for _ in range(10):
                nc.tensor.transpose(warm, ident128_bf, ident128_bf)
            def emit_fill(n):
                # dependency-free PE work: keeps the clock-gate/p-state up
                # during supply-paced idle; next matmul reloads weights anyway.
                for _ in range(n):
                    nc.tensor.ldweights(ident128_bf[:, :], is_transpose=True)

            # ---- prep ----
            QT = qkt.tile([W, S], bf16, tag="qt")
            KT = qkt.tile([W, S], bf16, tag="kt")
            VP = vpp.tile([128, n_sk, 65 * n_heads], bf16, tag="vp")
            nc.vector.memset(
                VP.rearrange("p c (h x) -> p c h x", x=65)[:, :, :, 64:65], 1.0
            )

            def emit_qk_prep(src, dstT, i4, eng, dma_eng=None):
                sl = slice(i4 * 512, (i4 + 1) * 512)
                rows = slice(i4 * 512, (i4 + 1) * 512)
                t_ld = ld.tile([128, 4, W], f32, tag="qk_ld", name=f"ld_{i4}")
                (dma_eng or nc.sync).dma_start(
                    out=t_ld,
                    in_=src[rows, :].rearrange("(u p) w -> p u w", p=128),
                )
                t_bf = ld.tile([128, 4, W], bf16, tag="qk_bf", name=f"bf_{i4}")
                eng.tensor_copy(t_bf, t_ld)
                tp = ps_act.tile([W, 512], bf16, tag="psa", name=f"tp_{i4}")
                for u in range(4):
                    nc.tensor.transpose(
                        tp[:, u * 128 : (u + 1) * 128], t_bf[:, u, :], ident128_bf
                    )
                nc.vector.tensor_copy(dstT[:, sl], tp)

            def emit_v_load(i4):
                rows = slice(i4 * 512, (i4 + 1) * 512)
                v_ld = ld.tile([128, 4, W], f32, tag="v_ld", name=f"vld_{i4}")
                nc.sync.dma_start(
                    out=v_ld,
                    in_=v_in[rows, :].rearrange("(u p) w -> p u w", p=128),
                )
                vdst = VP[:, i4 * 4 : (i4 + 1) * 4, :].rearrange(
                    "p u (h x) -> p u h x", x=65
                )[:, :, :, 0:64]
                vsrc = v_ld.rearrange("p u (h x) -> p u h x", x=64)
                (nc.gpsimd if i4 >= 2 else nc.vector).tensor_copy(vdst, vsrc)

            # up-front: only what the first steps need; the rest interleaves
            # into the early main-loop steps (kind, i4, emit_at_step).
            emit_qk_prep(q_in, QT, 0, nc.gpsimd)
            emit_qk_prep(k_in, KT, 0, nc.vector)
            emit_v_load(0)
            if n_sk > 4:
                emit_qk_prep(k_in, KT, 1, nc.vector)
                emit_v_load(1)
            pend_prep = []
            for i4 in range(2, n_sk // 4):
                pend_prep.append((max(0, 4 * i4 - 10), "k", i4))
                pend_prep.append((max(0, 4 * i4 - 8), "v", i4))
            pend_prep.sort()
            deferred_q = list(range(1, n_sk // 4))

            # ---- main pipeline ----
            steps = [(b, c) for b in range(n_blk) for c in range(n_sk)]
            ps_tiles = {}   # (b, c) -> act (t_h0, t_h1) | dve (t, None)

            def is_dve(idx):
                b, c = steps[idx]
                if S != _S:
                    return False
                return c in (dve_cs_b0 if b == 0 else dve_cs)

            def emit_qk(idx):
                b, c = steps[idx]
                t = ps_act.tile(
                    [128, 2 * blk], f32, tag="psa", name=f"psa_{b}_{c}"
                )
                ps_tiles[(b, c)] = t
                for h in range(n_heads):
                    p = h * 64
                    nc.tensor.matmul(
                        t[:, h * blk : (h + 1) * blk],
                        lhsT=KT[p : p + 64, c * 128 : (c + 1) * 128],
                        rhs=QT[p : p + 64, b * blk : (b + 1) * blk],
                        start=True,
                        stop=True,
                    )

            # ---- drain machinery ----
            drain_q = []
            osb_t = {}
            obm_t = {}

            def queue_drain(b, h, oT_tile):
                o_sb = osb.tile([65, blk], f16, tag="osb", name=f"osb_{h}_{b}")
                nc.vector.tensor_copy(o_sb, oT_tile)
                osb_t[(b, h)] = o_sb
                obm_t[(b, h)] = outb.tile(
                    [128, n_j, 64], f32, tag="obm", name=f"obm_{h}_{b}"
                )
                for j in range(n_j):
                    drain_q.append((b, h, j))

            def emit_drain_piece():
                b, h, j = drain_q.pop(0)
                o_sb = osb_t[(b, h)]
                obm = obm_t[(b, h)]
                tp2 = ps_act.tile([128, 65], f16, tag="psa", name=f"tp2_{b}_{h}_{j}")
                nc.tensor.transpose(tp2, o_sb[:, j * 128 : (j + 1) * 128], ident65)
                rec = small.tile([128, 1], f32, tag="rec", name=f"rec_{b}_{h}_{j}")
                nc.vector.reciprocal(rec, tp2[:, 64:65])
                nc.vector.tensor_scalar_mul(obm[:, j, :], tp2[:, 0:64], rec)
                if j == n_j - 1:
                    P0 = h * 64
                    nc.sync.dma_start(
                        out=out[b * blk : (b + 1) * blk, P0 : P0 + 64].rearrange(
                            "(j p) d -> p j d", p=128
                        ),
                        in_=obm,
                    )

            # ---- exp dispatch ----
            ex_of = {}      # (b, c, h) -> bf16 AP [128, blk]
            pend_p2 = []

            kI = -63.5 + 1.0 / 256.0  # I = round(J/128 + kI) == floor(y+64)

            def emit_exp(idx):
                b, c = steps[idx]
                t = ps_tiles.pop((b, c))
                if not is_dve(idx):
                    ex = expool.tile(
                        [128, 2 * blk], bf16, tag="ex", name=f"ex_{idx}"
                    )
                    nc.scalar.activation(
                        ex, t, mybir.ActivationFunctionType.Exp,
                        scale=_SCALE, bias=exp_bias,
                    )
                    for h in range(n_heads):
                        ex_of[(b, c, h)] = ex[:, h * blk : (h + 1) * blk]
                    return
                Jt = ijpool.tile([128, 2 * blk], u16, tag="J", name=f"J_{idx}")
                nc.vector.tensor_scalar(
                    out=Jt, in0=t, scalar1=ca, scalar2=c1b,
                    op0=mybir.AluOpType.mult, op1=mybir.AluOpType.add)
                It = ijpool.tile([128, 2 * blk], i16, tag="I", name=f"I_{idx}")
                nc.vector.tensor_scalar(
                    out=It, in0=Jt, scalar1=1.0 / 128.0, scalar2=kI,
                    op0=mybir.AluOpType.mult, op1=mybir.AluOpType.add)
                Pt = expool.tile([128, 2 * blk], u16, tag="ex", name=f"P_{idx}")

                def emit_p2(It=It, Jt=Jt, Pt=Pt, b=b, c=c):
                    nc.vector._custom_dve(
                        exp_op, out=Pt, in0=It, in1=Jt,
                        s0=128.0, s1=c2, imm2=-_GAMMA)
                    exb = Pt[:, :].bitcast(bf16)
                    for h in range(n_heads):
                        ex_of[(b, c, h)] = exb[:, h * blk : (h + 1) * blk]

                pend_p2.append((idx + 2, emit_p2))

            def flush_p2(now):
                while pend_p2 and pend_p2[0][0] <= now:
                    pend_p2.pop(0)[1]()

            # ---- PV queue ----
            pend_pv = []
            oT_t = {}
            pv_count = {}
            pv_started = {}

            def emit_pv(b, c, h):
                key = (b, h)
                if key not in oT_t:
                    oT_t[key] = ps_out.tile(
                        [65, blk], f32, tag=f"oT{h}", name=f"oT_{h}_{b}", bufs=1
                    )
                    pv_count[key] = 0
                    pv_started[key] = False
                oT = oT_t[key]
                ex = ex_of.pop((b, c, h))
                pv_count[key] += 1
                nc.tensor.matmul(
                    oT,
                    lhsT=VP[:, c, h * 65 : (h + 1) * 65],
                    rhs=ex,
                    start=not pv_started[key],
                    stop=(pv_count[key] == n_sk),
                )
                pv_started[key] = True
                if pv_count[key] == n_sk:
                    queue_drain(b, h, oT_t.pop(key))

            def flush_pv(now):
                i = 0
                while i < len(pend_pv):
                    if pend_pv[i][0] <= now:
                        _, b, c, h = pend_pv.pop(i)
                        emit_pv(b, c, h)
                    else:
                        i += 1

            # ---- main loop ----
            emit_qk(0)
            emit_qk(1)
            exp_done = set()

            def emit_exp_once(i):
                if i not in exp_done:
                    exp_done.add(i)
                    emit_exp(i)

            for idx, (b, c) in enumerate(steps):
                while pend_prep and pend_prep[0][0] <= idx:
                    _, kind, i4 = pend_prep.pop(0)
                    if kind == "k":
                        emit_qk_prep(k_in, KT, i4, nc.vector)
                    else:
                        emit_v_load(i4)
                if drain_q and idx % 4 == 1:
                    emit_drain_piece()
                emit_exp_once(idx)
                if idx + 1 < len(steps) and is_dve(idx + 1):
                    emit_exp_once(idx + 1)
                flush_p2(idx)
                flush_pv(idx)
                if idx + 2 < len(steps):
                    emit_qk(idx + 2)
                if deferred_q and deferred_q[0] == b + 1 and c == min(20, n_sk - 4):
                    emit_qk_prep(q_in, QT, deferred_q.pop(0), nc.gpsimd)
                lag = 5 if is_dve(idx) else 3
                if b == n_blk - 1 and c >= n_sk - 6:
                    lag = 2
                pend_pv.append((idx + lag, b, c, 0))
                h1_extra = 3 if c <= 1 else 0
                pend_pv.append((idx + lag + 1 + h1_extra, b, c, 1))
            tail = len(steps)
            while pend_p2 or pend_pv:
                flush_p2(tail)
                flush_pv(tail)
                for _ in range(min(2, len(drain_q))):
                    emit_drain_piece()
                tail += 1
            assert not deferred_q
            while drain_q:
                emit_drain_piece()
    nc.finalize()
    return nc


def _shard_inputs(query, key, value):
    w = _HPC * _D
    in_maps = []
    for c in range(_NCORES):
        sl = slice(c * w, (c + 1) * w)
        in_maps.append(
            {
                "q": np.ascontiguousarray(query[0, :, sl]),
                "k": np.ascontiguousarray(key[0, :, sl]),
                "v": np.ascontiguousarray(value[0, :, sl]),
            }
        )
    return in_maps


def kernel(query, key, value, trace=False, tmpdir=None):
    from concourse.bass_utils import run_bass_kernel_spmd

    query = np.asarray(query, dtype=np.float32)
    key = np.asarray(key, dtype=np.float32)
    value = np.asarray(value, dtype=np.float32)

    nc = build_program()
    in_maps = _shard_inputs(query, key, value)
    res = run_bass_kernel_spmd(
        nc, in_maps, list(range(_NCORES)), trace=trace, tmpdir=tmpdir
    )
    full = np.concatenate([res.results[c]["out"] for c in range(_NCORES)], axis=1)
    out = full[None].astype(np.float32)
    if trace:
        return out, res
    return out

